# revision 8
# baseline (speedup 1.0000x reference)
"""MoE routing kernel for Trainium2 (Bass/Tile), 8-core data-parallel.

Reference semantics (B=4096, D=1024, H=4096, E=8, GH=512, O=1024):
  gh     = relu(x @ gw1.T + gb1)            [B, GH]
  glog   = gh @ gw2.T + gb2                 [B, E]
  gate   = softmax(glog, axis=1)            [B, E]
  eh     = relu(einsum('bd,ehd->beh', x, ew1) + eb1)    [B, E, H]
  eo     = softmax(einsum('beh,eoh->beo', eh, ew2) + eb2, axis=2)
  idx    = argmax(log(gate) + gumbel(key42, (B, E)))    [B]  (== jax categorical)
  final  = eo[b, idx[b], :]                 [B, O]

Sharding: data-parallel over batch, 512 tokens per core; every core holds all
expert weights and computes its shard fully locally (no collectives).

Precision: gating network runs in fp32 (argmax/idx must match the reference
bit-for-bit in rank), expert MLPs run in float32r (full PE rate, ~1.5e-4 rms).
"""

import numpy as np
from contextlib import ExitStack

import concourse.bass as bass
import concourse.mybir as mybir
import concourse.tile as tile
from concourse import bacc
from concourse.bass_utils import run_bass_kernel_spmd

B, D, H, E, GH, O = 4096, 1024, 4096, 8, 512, 1024
NCORES = 8
BS = B // NCORES          # 512 tokens per core
NB = BS // 128            # 4 batch tiles per core

F32 = mybir.dt.float32
F32R = mybir.dt.float32r
I32 = mybir.dt.int32
U32 = mybir.dt.uint32
AF = mybir.ActivationFunctionType
ALU = mybir.AluOpType
AX = mybir.AxisListType


def _bcast128(ap2d):
    """[1, N] AP -> [128, N] AP broadcast along partitions (step-0)."""
    return bass.AP(tensor=ap2d.tensor, offset=ap2d.offset,
                   ap=[[0, 128]] + list(ap2d.ap)[1:])


def build(repeats: int = 1) -> bacc.Bacc:
    nc = bacc.Bacc(None)

    # ---- DRAM parameters (per-core shard views) ----
    xt = nc.declare_dram_parameter("xt", [D, BS], F32, isOutput=False)        # x shard, transposed
    gw1t = nc.declare_dram_parameter("gw1t", [D, GH], F32, isOutput=False)    # gw1.T
    gb1 = nc.declare_dram_parameter("gb1", [GH], F32, isOutput=False)
    gw2t = nc.declare_dram_parameter("gw2t", [GH, E], F32, isOutput=False)    # gw2.T
    gb2 = nc.declare_dram_parameter("gb2", [1, E], F32, isOutput=False)
    w1t = nc.declare_dram_parameter("w1t", [E, D, H], F32, isOutput=False)    # ew1 transposed per expert
    eb1 = nc.declare_dram_parameter("eb1", [E, H], F32, isOutput=False)
    w2t = nc.declare_dram_parameter("w2t", [E, H, O], F32, isOutput=False)    # ew2 transposed per expert
    eb2 = nc.declare_dram_parameter("eb2", [E, O], F32, isOutput=False)
    gum = nc.declare_dram_parameter("gum", [BS, E], F32, isOutput=False)      # gumbel noise shard

    final_d = nc.declare_dram_parameter("final", [BS, O], F32, isOutput=True)
    eo_d = nc.declare_dram_parameter("eo", [BS, E, O], F32, isOutput=True)
    gate_d = nc.declare_dram_parameter("gate", [BS, E], F32, isOutput=True)
    idx_d = nc.declare_dram_parameter("idx", [BS, 1], I32, isOutput=True)

    with ExitStack() as ctx:
        tc = ctx.enter_context(tile.TileContext(nc))

        def body():
            _emit_body(nc, tc, xt, gw1t, gb1, gw2t, gb2, w1t, eb1, w2t, eb2,
                       gum, final_d, eo_d, gate_d, idx_d)

        if repeats == 1:
            body()
        else:
            with tc.For_i(0, repeats, 1):
                body()

    nc.finalize()
    return nc


def _emit_body(nc, tc, xt, gw1t, gb1, gw2t, gb2, w1t, eb1, w2t, eb2,
               gum, final_d, eo_d, gate_d, idx_d):
    ctx = ExitStack()
    with ctx:
        # ---- resident pools (live through the whole body) ----
        res = ctx.enter_context(tc.tile_pool(name="res", bufs=1))
        small = ctx.enter_context(tc.tile_pool(name="small", bufs=4))

        # x^T shard, rounded to f32r for the expert matmuls (8 d-tiles of [128, BS])
        xr = []
        for dc in range(D // 128):
            t = res.tile([128, BS], F32R, tag=f"xr{dc}")
            nc.gpsimd.dma_start(out=t[:], in_=xt[dc * 128:(dc + 1) * 128, :])
            xr.append(t)

        # final accumulators (one per batch tile), zero-initialized
        final_acc = []
        for bt in range(NB):
            t = res.tile([128, O], F32, tag=f"fin{bt}")
            nc.vector.memset(t[:], 0.0)
            final_acc.append(t)

        # per-batch-tile one-hot(selected expert) [128, E]
        onehot = [res.tile([128, E], F32, tag=f"oh{bt}", name=f"oh{bt}") for bt in range(NB)]

        # =========================== GATING (fp32) ===========================
        with tc.tile_pool(name="gpool", bufs=1) as gpool, \
             tc.tile_pool(name="gps", bufs=2, space="PSUM") as gps:
            # fp32 copy of x^T for exact gating
            xf = []
            for dc in range(D // 128):
                t = gpool.tile([128, BS], F32, tag=f"xf{dc}")
                nc.sync.dma_start(out=t[:], in_=xt[dc * 128:(dc + 1) * 128, :])
                xf.append(t)
            # gw1t resident [D, GH] -> 8 tiles [128, GH]
            gw1_sb = []
            for dc in range(D // 128):
                t = gpool.tile([128, GH], F32, tag=f"gw1{dc}")
                nc.sync.dma_start(out=t[:], in_=gw1t[dc * 128:(dc + 1) * 128, :])
                gw1_sb.append(t)
            gw2_sb = gpool.tile([128, GH // 128, E], F32, tag="gw2")
            nc.sync.dma_start(
                out=gw2_sb[:],
                in_=gw2t[:].rearrange("(k p) e -> p k e", p=128))
            gb1_sb = gpool.tile([128, GH // 128], F32, tag="gb1")
            nc.sync.dma_start(out=gb1_sb[:],
                              in_=gb1[:].rearrange("(t p) -> p t", p=128))
            gb2_sb = gpool.tile([128, E], F32, tag="gb2")
            nc.sync.dma_start(
                out=gb2_sb[:],
in_=_bcast128(gb2[:, :]))

            # gating layer 1: gh[gh_tile, b] = relu(sum_d gw1t[d, gh] * xT[d, b])
            gh_sb = []
            for gt in range(GH // 128):
                psum = gps.tile([128, BS], F32, tag="gl1")
                for dc in range(D // 128):
                    nc.tensor.matmul(psum[:],
                                     gw1_sb[dc][:, gt * 128:(gt + 1) * 128],
                                     xf[dc][:],
                                     start=(dc == 0), stop=(dc == D // 128 - 1))
                t = gpool.tile([128, BS], F32, tag=f"gh{gt}")
                nc.scalar.activation(out=t[:], in_=psum[:], func=AF.Relu,
                                     bias=gb1_sb[:, gt:gt + 1], scale=1.0)
                gh_sb.append(t)

            # gating layer 2 + softmax + categorical, per batch tile
            for bt in range(NB):
                bsl = slice(bt * 128, (bt + 1) * 128)
                psum = gps.tile([128, E], F32, tag="gl2")
                for gt in range(GH // 128):
                    nc.tensor.matmul(psum[:], gh_sb[gt][:, bsl],
                                     gw2_sb[:, gt, :],
                                     start=(gt == 0), stop=(gt == GH // 128 - 1))
                nc.vector.tensor_add(psum[:], psum[:], gb2_sb[:])

                # gate = softmax(logits) along E
                negmax = small.tile([128, 1], F32, tag="negmax")
                nc.vector.reduce_max(negmax[:], psum[:], axis=AX.X, negate=True)
                gate_sb = small.tile([128, E], F32, tag="gate")
                rsum = small.tile([128, 1], F32, tag="rsum")
                nc.scalar.activation(out=gate_sb[:], in_=psum[:], func=AF.Exp,
                                     bias=negmax[:], scale=1.0,
                                     accum_out=rsum[:])
                rinv = small.tile([128, 1], F32, tag="rinv")
                nc.vector.reciprocal(rinv[:], rsum[:])
                nc.vector.tensor_scalar_mul(gate_sb[:], gate_sb[:], rinv[:])
                nc.sync.dma_start(out=gate_d[bsl, :], in_=gate_sb[:])

                # t = logits + gumbel ; idx = argmax_e t ; onehot = (t == max)
                gum_sb = small.tile([128, E], F32, tag="gum")
                nc.sync.dma_start(out=gum_sb[:], in_=gum[bsl, :])
                tvals = small.tile([128, E], F32, tag="tvals")
                nc.vector.tensor_add(tvals[:], psum[:], gum_sb[:])
                max8 = small.tile([128, 8], F32, tag="max8")
                nc.vector.max(max8[:], tvals[:])
                idx8 = small.tile([128, 8], U32, tag="idx8")
                nc.vector.max_index(idx8[:], max8[:], tvals[:])
                idx_i = small.tile([128, 1], I32, tag="idxi")
                nc.vector.tensor_copy(idx_i[:], idx8[:, 0:1])
                nc.sync.dma_start(out=idx_d[bsl, :], in_=idx_i[:])
                nc.vector.tensor_scalar(onehot[bt][:], tvals[:],
                                        max8[:, 0:1], None, op0=ALU.is_equal)

        # =========================== EXPERTS (f32r) ==========================
        NH = H // 128           # 32 h-tiles
        HB = H // 512           # 8 h-blocks (512 wide)
        NO = O // 512           # 2 o-blocks

        w1p = ctx.enter_context(tc.tile_pool(name="w1p", bufs=2))
        w2p = ctx.enter_context(tc.tile_pool(name="w2p", bufs=2))
        ehp = ctx.enter_context(tc.tile_pool(name="ehp", bufs=1))
        eop = ctx.enter_context(tc.tile_pool(name="eop", bufs=5))
        outp = ctx.enter_context(tc.tile_pool(name="outp", bufs=2))
        biasp = ctx.enter_context(tc.tile_pool(name="biasp", bufs=2))
        psA = ctx.enter_context(tc.tile_pool(name="psA", bufs=3, space="PSUM"))
        psB = ctx.enter_context(tc.tile_pool(name="psB", bufs=4, space="PSUM"))

        for e in range(E):
            # per-expert biases
            eb1_sb = biasp.tile([128, NH], F32, tag="eb1")
            nc.sync.dma_start(out=eb1_sb[:],
                              in_=eb1[e, :].rearrange("(t p) -> p t", p=128))
            eb2_sb = biasp.tile([128, O], F32, tag="eb2")
            nc.sync.dma_start(out=eb2_sb[:], in_=_bcast128(eb2[e:e + 1, :]))

            # ---- layer 1: eh[h, b] = relu(sum_d w1t[d, h] x[d, b]) ----
            eh_tiles = []
            for hb in range(HB):
                w1_sb = w1p.tile([128, D // 128, 512], F32R, tag="w1")
                nc.gpsimd.dma_start(
                    out=w1_sb[:].rearrange("p c h -> p c h"),
                    in_=w1t[e, :, hb * 512:(hb + 1) * 512]
                        .rearrange("(c p) h -> p c h", p=128))
                for ht in range(4):
                    h_idx = hb * 4 + ht
                    psum = psA.tile([128, BS], F32, tag="l1")
                    for dc in range(D // 128):
                        nc.tensor.matmul(psum[:],
                                         w1_sb[:, dc, ht * 128:(ht + 1) * 128],
                                         xr[dc][:],
                                         start=(dc == 0),
                                         stop=(dc == D // 128 - 1))
                    eh_t = ehp.tile([128, BS], F32R, tag=f"eh{h_idx}")
                    nc.scalar.activation(out=eh_t[:], in_=psum[:], func=AF.Relu,
                                         bias=eb1_sb[:, h_idx:h_idx + 1],
                                         scale=1.0)
                    eh_tiles.append(eh_t)

            # ---- layer 2 + bias: eo_pre[b, o] = sum_h eh[h, b] w2t[h, o] ----
            eo_pre = [eop.tile([128, O], F32, tag="eo_pre", name=f"eo_pre{i}") for i in range(NB)]
            for ob in range(NO):
                osl = slice(ob * 512, (ob + 1) * 512)
                psums = [psB.tile([128, 512], F32, tag="l2", name=f"l2ps{i}") for i in range(NB)]
                for hg in range(HB):
                    w2_sb = w2p.tile([128, 4, 512], F32R, tag="w2")
                    nc.gpsimd.dma_start(
                        out=w2_sb[:],
                        in_=w2t[e, hg * 512:(hg + 1) * 512, osl]
                            .rearrange("(c p) o -> p c o", p=128))
                    for bt in range(NB):
                        bsl = slice(bt * 128, (bt + 1) * 128)
                        for hc in range(4):
                            nc.tensor.matmul(psums[bt][:],
                                             eh_tiles[hg * 4 + hc][:, bsl],
                                             w2_sb[:, hc, :],
                                             start=(hg == 0 and hc == 0),
                                             stop=(hg == HB - 1 and hc == 3))
                for bt in range(NB):
                    nc.vector.tensor_add(eo_pre[bt][:, osl], psums[bt][:],
                                         eb2_sb[:, osl])

            # ---- softmax over O + masked accumulation into final ----
            for bt in range(NB):
                bsl = slice(bt * 128, (bt + 1) * 128)
                negmax = small.tile([128, 1], F32, tag="negmax2")
                nc.vector.reduce_max(negmax[:], eo_pre[bt][:], axis=AX.X,
                                     negate=True)
                rsum = small.tile([128, 1], F32, tag="rsum2")
                nc.scalar.activation(out=eo_pre[bt][:], in_=eo_pre[bt][:],
                                     func=AF.Exp, bias=negmax[:], scale=1.0,
                                     accum_out=rsum[:])
                rinv = small.tile([128, 1], F32, tag="rinv2")
                nc.vector.reciprocal(rinv[:], rsum[:])
                eo_n = outp.tile([128, O], F32, tag="eo_n")
                nc.vector.tensor_scalar_mul(eo_n[:], eo_pre[bt][:], rinv[:])
                nc.sync.dma_start(out=eo_d[bsl, e, :], in_=eo_n[:])
                # final += onehot[:, e] * eo_n   (reuse eo_pre as scratch)
                nc.vector.tensor_scalar_mul(eo_pre[bt][:], eo_n[:],
                                            onehot[bt][:, e:e + 1])
                nc.vector.tensor_add(final_acc[bt][:], final_acc[bt][:],
                                     eo_pre[bt][:])

        for bt in range(NB):
            nc.sync.dma_start(out=final_d[bt * 128:(bt + 1) * 128, :],
                              in_=final_acc[bt][:])


_NC_CACHE = {}


def _get_nc(repeats: int = 1):
    if repeats not in _NC_CACHE:
        _NC_CACHE[repeats] = build(repeats)
    return _NC_CACHE[repeats]


def _gumbel_noise() -> np.ndarray:
    # Must reproduce jax.random.categorical(jax.random.key(42), ...) noise on
    # the same jax backend the reference runs on (RNG bits are backend-
    # dependent here), so use the default backend.
    import jax
    import jax.numpy as jnp
    g = jax.random.gumbel(jax.random.key(42), (B, E), jnp.float32)
    return np.asarray(g)


def _prep_inputs(inputs: dict) -> list[dict]:
    x = np.asarray(inputs["x"], np.float32)
    gw1 = np.asarray(inputs["gw1"], np.float32)
    gb1 = np.asarray(inputs["gb1"], np.float32)
    gw2 = np.asarray(inputs["gw2"], np.float32)
    gb2 = np.asarray(inputs["gb2"], np.float32)
    ew1 = np.asarray(inputs["ew1"], np.float32)
    eb1 = np.asarray(inputs["eb1"], np.float32)
    ew2 = np.asarray(inputs["ew2"], np.float32)
    eb2 = np.asarray(inputs["eb2"], np.float32)

    gw1t = np.ascontiguousarray(gw1.T)                # [D, GH]
    gw2t = np.ascontiguousarray(gw2.T)                # [GH, E]
    w1t = np.ascontiguousarray(ew1.transpose(0, 2, 1))  # [E, D, H]
    w2t = np.ascontiguousarray(ew2.transpose(0, 2, 1))  # [E, H, O]
    gum = _gumbel_noise()                             # [B, E]

    in_maps = []
    for c in range(NCORES):
        bsl = slice(c * BS, (c + 1) * BS)
        in_maps.append({
            "xt": np.ascontiguousarray(x[bsl].T),     # [D, BS]
            "gw1t": gw1t, "gb1": gb1, "gw2t": gw2t,
            "gb2": gb2.reshape(1, E),
            "w1t": w1t, "eb1": eb1, "w2t": w2t, "eb2": eb2,
            "gum": np.ascontiguousarray(gum[bsl]),
        })
    return in_maps


def run(inputs: dict, repeats: int = 1):
    nc = _get_nc(repeats)
    in_maps = _prep_inputs(inputs)
    res = run_bass_kernel_spmd(nc, in_maps, list(range(NCORES)))
    final = np.concatenate([r["final"] for r in res.results], axis=0)
    eo = np.concatenate([r["eo"] for r in res.results], axis=0)
    gate = np.concatenate([r["gate"] for r in res.results], axis=0)
    idx = np.concatenate([r["idx"][:, 0] for r in res.results], axis=0)
    return final, eo, gate, idx.astype(np.int32)


def kernel(**inputs):
    return run(inputs, repeats=1)


# revision 11
# speedup vs baseline: 1.0060x; 1.0060x over previous
"""MoE routing kernel for Trainium2 (Bass/Tile), 8-core data-parallel.

Reference semantics (B=4096, D=1024, H=4096, E=8, GH=512, O=1024):
  gh     = relu(x @ gw1.T + gb1)            [B, GH]
  glog   = gh @ gw2.T + gb2                 [B, E]
  gate   = softmax(glog, axis=1)            [B, E]
  eh     = relu(einsum('bd,ehd->beh', x, ew1) + eb1)    [B, E, H]
  eo     = softmax(einsum('beh,eoh->beo', eh, ew2) + eb2, axis=2)
  idx    = argmax(log(gate) + gumbel(key42, (B, E)))    [B]  (== jax categorical)
  final  = eo[b, idx[b], :]                 [B, O]

Sharding: data-parallel over batch, 512 tokens per core; every core holds all
expert weights and computes its shard fully locally (no collectives).

Precision: gating network runs in fp32 (argmax/idx must match the reference
bit-for-bit in rank), expert MLPs run in float32r (full PE rate, ~1.5e-4 rms).
"""

import numpy as np
from contextlib import ExitStack

import concourse.bass as bass
import concourse.mybir as mybir
import concourse.tile as tile
from concourse import bacc
from concourse.bass_utils import run_bass_kernel_spmd

B, D, H, E, GH, O = 4096, 1024, 4096, 8, 512, 1024
NCORES = 8
BS = B // NCORES          # 512 tokens per core
NB = BS // 128            # 4 batch tiles per core

F32 = mybir.dt.float32
F32R = mybir.dt.float32r
I32 = mybir.dt.int32
U32 = mybir.dt.uint32
AF = mybir.ActivationFunctionType
ALU = mybir.AluOpType
AX = mybir.AxisListType


def _bcast128(ap2d):
    """[1, N] AP -> [128, N] AP broadcast along partitions (step-0)."""
    return bass.AP(tensor=ap2d.tensor, offset=ap2d.offset,
                   ap=[[0, 128]] + list(ap2d.ap)[1:])


def build(repeats: int = 1, variant: str = "full") -> bacc.Bacc:
    nc = bacc.Bacc(None)

    # ---- DRAM parameters (per-core shard views) ----
    xt = nc.declare_dram_parameter("xt", [D, BS], F32, isOutput=False)        # x shard, transposed
    gw1t = nc.declare_dram_parameter("gw1t", [D, GH], F32, isOutput=False)    # gw1.T
    gb1 = nc.declare_dram_parameter("gb1", [GH], F32, isOutput=False)
    gw2t = nc.declare_dram_parameter("gw2t", [GH, E], F32, isOutput=False)    # gw2.T
    gb2 = nc.declare_dram_parameter("gb2", [1, E], F32, isOutput=False)
    w1t = nc.declare_dram_parameter("w1t", [E, D, H], F32, isOutput=False)    # ew1 transposed per expert
    eb1 = nc.declare_dram_parameter("eb1", [E, H], F32, isOutput=False)
    w2t = nc.declare_dram_parameter("w2t", [E, H, O], F32, isOutput=False)    # ew2 transposed per expert
    eb2 = nc.declare_dram_parameter("eb2", [E, O], F32, isOutput=False)
    gum = nc.declare_dram_parameter("gum", [BS, E], F32, isOutput=False)      # gumbel noise shard

    final_d = nc.declare_dram_parameter("final", [BS, O], F32, isOutput=True)
    eo_d = nc.declare_dram_parameter("eo", [BS, E, O], F32, isOutput=True)
    gate_d = nc.declare_dram_parameter("gate", [BS, E], F32, isOutput=True)
    idx_d = nc.declare_dram_parameter("idx", [BS, 1], I32, isOutput=True)

    with ExitStack() as ctx:
        tc = ctx.enter_context(tile.TileContext(nc))

        def body():
            _emit_body(nc, tc, xt, gw1t, gb1, gw2t, gb2, w1t, eb1, w2t, eb2,
                       gum, final_d, eo_d, gate_d, idx_d, variant)

        if repeats == 1:
            body()
        else:
            with tc.For_i(0, repeats, 1):
                body()

    nc.finalize()
    return nc


def _emit_body(nc, tc, xt, gw1t, gb1, gw2t, gb2, w1t, eb1, w2t, eb2,
               gum, final_d, eo_d, gate_d, idx_d, variant="full"):
    ctx = ExitStack()
    with ctx:
        # ---- resident pools (live through the whole body) ----
        res = ctx.enter_context(tc.tile_pool(name="res", bufs=1))
        small = ctx.enter_context(tc.tile_pool(name="small", bufs=4))

        # x^T shard, rounded to f32r for the expert matmuls (8 d-tiles of [128, BS])
        xr = []
        for dc in range(D // 128):
            t = res.tile([128, BS], F32R, tag=f"xr{dc}")
            nc.gpsimd.dma_start(out=t[:], in_=xt[dc * 128:(dc + 1) * 128, :])
            xr.append(t)

        # final accumulators (one per batch tile), zero-initialized
        final_acc = []
        for bt in range(NB):
            t = res.tile([128, O], F32, tag=f"fin{bt}")
            nc.vector.memset(t[:], 0.0)
            final_acc.append(t)

        # per-batch-tile one-hot(selected expert) [128, E]
        onehot = [res.tile([128, E], F32, tag=f"oh{bt}", name=f"oh{bt}") for bt in range(NB)]

        # =========================== GATING (fp32) ===========================
        with tc.tile_pool(name="gpool", bufs=1) as gpool, \
             tc.tile_pool(name="gps", bufs=2, space="PSUM") as gps:
            # fp32 copy of x^T for exact gating
            xf = []
            for dc in range(D // 128):
                t = gpool.tile([128, BS], F32, tag=f"xf{dc}")
                nc.sync.dma_start(out=t[:], in_=xt[dc * 128:(dc + 1) * 128, :])
                xf.append(t)
            # gw1t resident [D, GH] -> 8 tiles [128, GH]
            gw1_sb = []
            for dc in range(D // 128):
                t = gpool.tile([128, GH], F32, tag=f"gw1{dc}")
                nc.sync.dma_start(out=t[:], in_=gw1t[dc * 128:(dc + 1) * 128, :])
                gw1_sb.append(t)
            gw2_sb = gpool.tile([128, GH // 128, E], F32, tag="gw2")
            nc.sync.dma_start(
                out=gw2_sb[:],
                in_=gw2t[:].rearrange("(k p) e -> p k e", p=128))
            gb1_sb = gpool.tile([128, GH // 128], F32, tag="gb1")
            nc.sync.dma_start(out=gb1_sb[:],
                              in_=gb1[:].rearrange("(t p) -> p t", p=128))
            gb2_sb = gpool.tile([128, E], F32, tag="gb2")
            nc.sync.dma_start(
                out=gb2_sb[:],
in_=_bcast128(gb2[:, :]))

            # gating layer 1: gh[gh_tile, b] = relu(sum_d gw1t[d, gh] * xT[d, b])
            gh_sb = []
            for gt in range(GH // 128):
                psum = gps.tile([128, BS], F32, tag="gl1")
                for dc in range(D // 128):
                    nc.tensor.matmul(psum[:],
                                     gw1_sb[dc][:, gt * 128:(gt + 1) * 128],
                                     xf[dc][:],
                                     start=(dc == 0), stop=(dc == D // 128 - 1))
                t = gpool.tile([128, BS], F32, tag=f"gh{gt}")
                nc.scalar.activation(out=t[:], in_=psum[:], func=AF.Relu,
                                     bias=gb1_sb[:, gt:gt + 1], scale=1.0)
                gh_sb.append(t)

            # gating layer 2 + softmax + categorical, per batch tile
            for bt in range(NB):
                bsl = slice(bt * 128, (bt + 1) * 128)
                psum = gps.tile([128, E], F32, tag="gl2")
                for gt in range(GH // 128):
                    nc.tensor.matmul(psum[:], gh_sb[gt][:, bsl],
                                     gw2_sb[:, gt, :],
                                     start=(gt == 0), stop=(gt == GH // 128 - 1))
                nc.vector.tensor_add(psum[:], psum[:], gb2_sb[:])

                # gate = softmax(logits) along E
                negmax = small.tile([128, 1], F32, tag="negmax")
                nc.vector.reduce_max(negmax[:], psum[:], axis=AX.X, negate=True)
                gate_sb = small.tile([128, E], F32, tag="gate")
                rsum = small.tile([128, 1], F32, tag="rsum")
                nc.scalar.activation(out=gate_sb[:], in_=psum[:], func=AF.Exp,
                                     bias=negmax[:], scale=1.0,
                                     accum_out=rsum[:])
                rinv = small.tile([128, 1], F32, tag="rinv")
                nc.vector.reciprocal(rinv[:], rsum[:])
                nc.vector.tensor_scalar_mul(gate_sb[:], gate_sb[:], rinv[:])
                nc.sync.dma_start(out=gate_d[bsl, :], in_=gate_sb[:])

                # t = logits + gumbel ; idx = argmax_e t ; onehot = (t == max)
                gum_sb = small.tile([128, E], F32, tag="gum")
                nc.sync.dma_start(out=gum_sb[:], in_=gum[bsl, :])
                tvals = small.tile([128, E], F32, tag="tvals")
                nc.vector.tensor_add(tvals[:], psum[:], gum_sb[:])
                max8 = small.tile([128, 8], F32, tag="max8")
                nc.vector.max(max8[:], tvals[:])
                idx8 = small.tile([128, 8], U32, tag="idx8")
                nc.vector.max_index(idx8[:], max8[:], tvals[:])
                idx_i = small.tile([128, 1], I32, tag="idxi")
                nc.vector.tensor_copy(idx_i[:], idx8[:, 0:1])
                nc.sync.dma_start(out=idx_d[bsl, :], in_=idx_i[:])
                nc.vector.tensor_scalar(onehot[bt][:], tvals[:],
                                        max8[:, 0:1], None, op0=ALU.is_equal)

        # =========================== EXPERTS (f32r) ==========================
        NH = H // 128           # 32 h-tiles
        HB = H // 512           # 8 h-blocks (512 wide)
        NO = O // 512           # 2 o-blocks

        w1p = ctx.enter_context(tc.tile_pool(name="w1p", bufs=2))
        w2p = ctx.enter_context(tc.tile_pool(name="w2p", bufs=2))
        ehp = ctx.enter_context(tc.tile_pool(name="ehp", bufs=1))
        eop = ctx.enter_context(tc.tile_pool(name="eop", bufs=5))
        outp = ctx.enter_context(tc.tile_pool(name="outp", bufs=2))
        biasp = ctx.enter_context(tc.tile_pool(name="biasp", bufs=2))
        psA = ctx.enter_context(tc.tile_pool(name="psA", bufs=3, space="PSUM"))
        psB = ctx.enter_context(tc.tile_pool(name="psB", bufs=4, space="PSUM"))

        for e in range(E):
            # per-expert biases
            eb1_sb = biasp.tile([128, NH], F32, tag="eb1")
            nc.sync.dma_start(out=eb1_sb[:],
                              in_=eb1[e, :].rearrange("(t p) -> p t", p=128))
            eb2_sb = biasp.tile([128, O], F32, tag="eb2")
            nc.sync.dma_start(out=eb2_sb[:], in_=_bcast128(eb2[e:e + 1, :]))

            # ---- layer 1: eh[h, b] = relu(sum_d w1t[d, h] x[d, b]) ----
            eh_tiles = []
            for hb in range(HB):
                w1_sb = w1p.tile([128, D // 128, 512], F32R, tag="w1")
                if variant == "nodma":
                    nc.gpsimd.dma_start(
                        out=w1_sb[:, 0:1, 0:32],
                        in_=w1t[e, 0:128, hb * 512:hb * 512 + 32]
                            .rearrange("(c p) h -> p c h", p=128))
                else:
                    nc.gpsimd.dma_start(
                        out=w1_sb[:],
                        in_=w1t[e, :, hb * 512:(hb + 1) * 512]
                            .rearrange("(c p) h -> p c h", p=128))
                for ht in range(4):
                    h_idx = hb * 4 + ht
                    if variant != "nomm":
                        psum = psA.tile([128, BS], F32, tag="l1")
                        for dc in range(D // 128):
                            nc.tensor.matmul(psum[:],
                                             w1_sb[:, dc, ht * 128:(ht + 1) * 128],
                                             xr[dc][:],
                                             start=(dc == 0),
                                             stop=(dc == D // 128 - 1))
                        act_in = psum[:]
                    else:
                        act_in = w1_sb[:, 0, :]
                    eh_t = ehp.tile([128, BS], F32R, tag=f"eh{h_idx}")
                    nc.scalar.activation(out=eh_t[:], in_=act_in, func=AF.Relu,
                                         bias=eb1_sb[:, h_idx:h_idx + 1],
                                         scale=1.0)
                    eh_tiles.append(eh_t)

            # ---- layer 2 + bias: eo_pre[b, o] = sum_h eh[h, b] w2t[h, o] ----
            eo_pre = [eop.tile([128, O], F32, tag="eo_pre", name=f"eo_pre{i}") for i in range(NB)]
            for ob in range(NO):
                osl = slice(ob * 512, (ob + 1) * 512)
                psums = ([psB.tile([128, 512], F32, tag="l2", name=f"l2ps{i}") for i in range(NB)]
                         if variant != "nomm" else None)
                for hg in range(HB):
                    w2_sb = w2p.tile([128, 4, 512], F32R, tag="w2")
                    if variant == "nodma":
                        nc.gpsimd.dma_start(
                            out=w2_sb[:, 0:1, 0:32],
                            in_=w2t[e, hg * 512:hg * 512 + 128,
                                    ob * 512:ob * 512 + 32]
                                .rearrange("(c p) o -> p c o", p=128))
                    else:
                        nc.gpsimd.dma_start(
                            out=w2_sb[:],
                            in_=w2t[e, hg * 512:(hg + 1) * 512, osl]
                                .rearrange("(c p) o -> p c o", p=128))
                    for bt in range(NB):
                        bsl = slice(bt * 128, (bt + 1) * 128)
                        for hc in range(4):
                            if variant == "nomm":
                                continue
                            nc.tensor.matmul(psums[bt][:],
                                             eh_tiles[hg * 4 + hc][:, bsl],
                                             w2_sb[:, hc, :],
                                             start=(hg == 0 and hc == 0),
                                             stop=(hg == HB - 1 and hc == 3))
                for bt in range(NB):
                    drain_in = (psums[bt][:] if variant != "nomm"
                                else eh_tiles[bt][:, 0:512].bitcast(F32))
                    nc.vector.tensor_add(eo_pre[bt][:, osl], drain_in,
                                         eb2_sb[:, osl])

            # ---- softmax over O + masked accumulation into final ----
            for bt in range(NB if variant != "nopost" else 0):
                bsl = slice(bt * 128, (bt + 1) * 128)
                negmax = small.tile([128, 1], F32, tag="negmax2")
                nc.vector.reduce_max(negmax[:], eo_pre[bt][:], axis=AX.X,
                                     negate=True)
                rsum = small.tile([128, 1], F32, tag="rsum2")
                nc.scalar.activation(out=eo_pre[bt][:], in_=eo_pre[bt][:],
                                     func=AF.Exp, bias=negmax[:], scale=1.0,
                                     accum_out=rsum[:])
                rinv = small.tile([128, 1], F32, tag="rinv2")
                nc.vector.reciprocal(rinv[:], rsum[:])
                eo_n = outp.tile([128, O], F32, tag="eo_n")
                nc.vector.tensor_scalar_mul(eo_n[:], eo_pre[bt][:], rinv[:])
                nc.sync.dma_start(out=eo_d[bsl, e, :], in_=eo_n[:])
                # final += onehot[:, e] * eo_n   (reuse eo_pre as scratch)
                nc.vector.tensor_scalar_mul(eo_pre[bt][:], eo_n[:],
                                            onehot[bt][:, e:e + 1])
                nc.vector.tensor_add(final_acc[bt][:], final_acc[bt][:],
                                     eo_pre[bt][:])

        for bt in range(NB):
            nc.sync.dma_start(out=final_d[bt * 128:(bt + 1) * 128, :],
                              in_=final_acc[bt][:])


_NC_CACHE = {}


def _get_nc(repeats: int = 1):
    if repeats not in _NC_CACHE:
        _NC_CACHE[repeats] = build(repeats)
    return _NC_CACHE[repeats]


def _gumbel_noise() -> np.ndarray:
    # Must reproduce jax.random.categorical(jax.random.key(42), ...) noise on
    # the same jax backend the reference runs on (RNG bits are backend-
    # dependent here), so use the default backend.
    import jax
    import jax.numpy as jnp
    g = jax.random.gumbel(jax.random.key(42), (B, E), jnp.float32)
    return np.asarray(g)


def _prep_inputs(inputs: dict) -> list[dict]:
    x = np.asarray(inputs["x"], np.float32)
    gw1 = np.asarray(inputs["gw1"], np.float32)
    gb1 = np.asarray(inputs["gb1"], np.float32)
    gw2 = np.asarray(inputs["gw2"], np.float32)
    gb2 = np.asarray(inputs["gb2"], np.float32)
    ew1 = np.asarray(inputs["ew1"], np.float32)
    eb1 = np.asarray(inputs["eb1"], np.float32)
    ew2 = np.asarray(inputs["ew2"], np.float32)
    eb2 = np.asarray(inputs["eb2"], np.float32)

    gw1t = np.ascontiguousarray(gw1.T)                # [D, GH]
    gw2t = np.ascontiguousarray(gw2.T)                # [GH, E]
    w1t = np.ascontiguousarray(ew1.transpose(0, 2, 1))  # [E, D, H]
    w2t = np.ascontiguousarray(ew2.transpose(0, 2, 1))  # [E, H, O]
    gum = _gumbel_noise()                             # [B, E]

    in_maps = []
    for c in range(NCORES):
        bsl = slice(c * BS, (c + 1) * BS)
        in_maps.append({
            "xt": np.ascontiguousarray(x[bsl].T),     # [D, BS]
            "gw1t": gw1t, "gb1": gb1, "gw2t": gw2t,
            "gb2": gb2.reshape(1, E),
            "w1t": w1t, "eb1": eb1, "w2t": w2t, "eb2": eb2,
            "gum": np.ascontiguousarray(gum[bsl]),
        })
    return in_maps


def run(inputs: dict, repeats: int = 1):
    nc = _get_nc(repeats)
    in_maps = _prep_inputs(inputs)
    res = run_bass_kernel_spmd(nc, in_maps, list(range(NCORES)))
    final = np.concatenate([r["final"] for r in res.results], axis=0)
    eo = np.concatenate([r["eo"] for r in res.results], axis=0)
    gate = np.concatenate([r["gate"] for r in res.results], axis=0)
    idx = np.concatenate([r["idx"][:, 0] for r in res.results], axis=0)
    return final, eo, gate, idx.astype(np.int32)


def kernel(**inputs):
    return run(inputs, repeats=1)


# revision 12
# speedup vs baseline: 1.9747x; 1.9629x over previous
"""MoE routing kernel for Trainium2 (Bass/Tile), 8-core data-parallel.

Reference semantics (B=4096, D=1024, H=4096, E=8, GH=512, O=1024):
  gh     = relu(x @ gw1.T + gb1)            [B, GH]
  glog   = gh @ gw2.T + gb2                 [B, E]
  gate   = softmax(glog, axis=1)            [B, E]
  eh     = relu(einsum('bd,ehd->beh', x, ew1) + eb1)    [B, E, H]
  eo     = softmax(einsum('beh,eoh->beo', eh, ew2) + eb2, axis=2)
  idx    = argmax(log(gate) + gumbel(key42, (B, E)))    [B]  (== jax categorical)
  final  = eo[b, idx[b], :]                 [B, O]

Sharding: data-parallel over batch, 512 tokens per core; every core holds all
expert weights and computes its shard fully locally (no collectives).

Precision: gating network runs in fp32 (argmax/idx must match the reference
bit-for-bit in rank), expert MLPs run in float32r (full PE rate, ~1.5e-4 rms).
"""

import numpy as np
from contextlib import ExitStack

import concourse.bass as bass
import concourse.mybir as mybir
import concourse.tile as tile
from concourse import bacc
from concourse.bass_utils import run_bass_kernel_spmd

B, D, H, E, GH, O = 4096, 1024, 4096, 8, 512, 1024
NCORES = 8
BS = B // NCORES          # 512 tokens per core
NB = BS // 128            # 4 batch tiles per core

F32 = mybir.dt.float32
F32R = mybir.dt.float32r
I32 = mybir.dt.int32
U32 = mybir.dt.uint32
AF = mybir.ActivationFunctionType
ALU = mybir.AluOpType
AX = mybir.AxisListType


def _bcast128(ap2d):
    """[1, N] AP -> [128, N] AP broadcast along partitions (step-0)."""
    return bass.AP(tensor=ap2d.tensor, offset=ap2d.offset,
                   ap=[[0, 128]] + list(ap2d.ap)[1:])


def build(repeats: int = 1, variant: str = "full") -> bacc.Bacc:
    nc = bacc.Bacc(None)

    # ---- DRAM parameters (per-core shard views) ----
    xt = nc.declare_dram_parameter("xt", [D, BS], F32, isOutput=False)        # x shard, transposed
    gw1t = nc.declare_dram_parameter("gw1t", [D, GH], F32, isOutput=False)    # gw1.T
    gb1 = nc.declare_dram_parameter("gb1", [GH], F32, isOutput=False)
    gw2t = nc.declare_dram_parameter("gw2t", [GH, E], F32, isOutput=False)    # gw2.T
    gb2 = nc.declare_dram_parameter("gb2", [1, E], F32, isOutput=False)
    w1t = nc.declare_dram_parameter("w1t", [E, D, H], F32, isOutput=False)    # ew1 transposed per expert
    eb1 = nc.declare_dram_parameter("eb1", [E, H], F32, isOutput=False)
    w2t = nc.declare_dram_parameter("w2t", [E, H, O], F32, isOutput=False)    # ew2 transposed per expert
    eb2 = nc.declare_dram_parameter("eb2", [E, O], F32, isOutput=False)
    gum = nc.declare_dram_parameter("gum", [BS, E], F32, isOutput=False)      # gumbel noise shard

    final_d = nc.declare_dram_parameter("final", [BS, O], F32, isOutput=True)
    eo_d = nc.declare_dram_parameter("eo", [BS, E, O], F32, isOutput=True)
    gate_d = nc.declare_dram_parameter("gate", [BS, E], F32, isOutput=True)
    idx_d = nc.declare_dram_parameter("idx", [BS, 1], I32, isOutput=True)

    with ExitStack() as ctx:
        tc = ctx.enter_context(tile.TileContext(nc))

        def body():
            _emit_body(nc, tc, xt, gw1t, gb1, gw2t, gb2, w1t, eb1, w2t, eb2,
                       gum, final_d, eo_d, gate_d, idx_d, variant)

        if repeats == 1:
            body()
        else:
            with tc.For_i(0, repeats, 1):
                body()

    nc.finalize()
    return nc


def _emit_body(nc, tc, xt, gw1t, gb1, gw2t, gb2, w1t, eb1, w2t, eb2,
               gum, final_d, eo_d, gate_d, idx_d, variant="full"):
    ctx = ExitStack()
    with ctx:
        # ---- resident pools (live through the whole body) ----
        res = ctx.enter_context(tc.tile_pool(name="res", bufs=1))
        small = ctx.enter_context(tc.tile_pool(name="small", bufs=4))

        # x^T shard, rounded to f32r for the expert matmuls (8 d-tiles of [128, BS])
        xr = []
        for dc in range(D // 128):
            t = res.tile([128, BS], F32R, tag=f"xr{dc}")
            nc.gpsimd.dma_start(out=t[:], in_=xt[dc * 128:(dc + 1) * 128, :])
            xr.append(t)

        # final accumulators (one per batch tile), zero-initialized
        final_acc = []
        for bt in range(NB):
            t = res.tile([128, O], F32, tag=f"fin{bt}")
            nc.vector.memset(t[:], 0.0)
            final_acc.append(t)

        # per-batch-tile one-hot(selected expert) [128, E]
        onehot = [res.tile([128, E], F32, tag=f"oh{bt}", name=f"oh{bt}") for bt in range(NB)]

        # =========================== GATING (fp32) ===========================
        with tc.tile_pool(name="gpool", bufs=1) as gpool, \
             tc.tile_pool(name="gps", bufs=2, space="PSUM") as gps:
            # fp32 copy of x^T for exact gating
            xf = []
            for dc in range(D // 128):
                t = gpool.tile([128, BS], F32, tag=f"xf{dc}")
                nc.sync.dma_start(out=t[:], in_=xt[dc * 128:(dc + 1) * 128, :])
                xf.append(t)
            # gw1t resident [D, GH] -> 8 tiles [128, GH]
            gw1_sb = []
            for dc in range(D // 128):
                t = gpool.tile([128, GH], F32, tag=f"gw1{dc}")
                nc.sync.dma_start(out=t[:], in_=gw1t[dc * 128:(dc + 1) * 128, :])
                gw1_sb.append(t)
            gw2_sb = gpool.tile([128, GH // 128, E], F32, tag="gw2")
            nc.sync.dma_start(
                out=gw2_sb[:],
                in_=gw2t[:].rearrange("(k p) e -> p k e", p=128))
            gb1_sb = gpool.tile([128, GH // 128], F32, tag="gb1")
            nc.sync.dma_start(out=gb1_sb[:],
                              in_=gb1[:].rearrange("(t p) -> p t", p=128))
            gb2_sb = gpool.tile([128, E], F32, tag="gb2")
            nc.sync.dma_start(
                out=gb2_sb[:],
in_=_bcast128(gb2[:, :]))

            # gating layer 1: gh[gh_tile, b] = relu(sum_d gw1t[d, gh] * xT[d, b])
            gh_sb = []
            for gt in range(GH // 128):
                psum = gps.tile([128, BS], F32, tag="gl1")
                for dc in range(D // 128):
                    nc.tensor.matmul(psum[:],
                                     gw1_sb[dc][:, gt * 128:(gt + 1) * 128],
                                     xf[dc][:],
                                     start=(dc == 0), stop=(dc == D // 128 - 1))
                t = gpool.tile([128, BS], F32, tag=f"gh{gt}")
                nc.scalar.activation(out=t[:], in_=psum[:], func=AF.Relu,
                                     bias=gb1_sb[:, gt:gt + 1], scale=1.0)
                gh_sb.append(t)

            # gating layer 2 + softmax + categorical, per batch tile
            for bt in range(NB):
                bsl = slice(bt * 128, (bt + 1) * 128)
                psum = gps.tile([128, E], F32, tag="gl2")
                for gt in range(GH // 128):
                    nc.tensor.matmul(psum[:], gh_sb[gt][:, bsl],
                                     gw2_sb[:, gt, :],
                                     start=(gt == 0), stop=(gt == GH // 128 - 1))
                nc.vector.tensor_add(psum[:], psum[:], gb2_sb[:])

                # gate = softmax(logits) along E
                negmax = small.tile([128, 1], F32, tag="negmax")
                nc.vector.reduce_max(negmax[:], psum[:], axis=AX.X, negate=True)
                gate_sb = small.tile([128, E], F32, tag="gate")
                rsum = small.tile([128, 1], F32, tag="rsum")
                nc.scalar.activation(out=gate_sb[:], in_=psum[:], func=AF.Exp,
                                     bias=negmax[:], scale=1.0,
                                     accum_out=rsum[:])
                rinv = small.tile([128, 1], F32, tag="rinv")
                nc.vector.reciprocal(rinv[:], rsum[:])
                nc.vector.tensor_scalar_mul(gate_sb[:], gate_sb[:], rinv[:])
                nc.sync.dma_start(out=gate_d[bsl, :], in_=gate_sb[:])

                # t = logits + gumbel ; idx = argmax_e t ; onehot = (t == max)
                gum_sb = small.tile([128, E], F32, tag="gum")
                nc.sync.dma_start(out=gum_sb[:], in_=gum[bsl, :])
                tvals = small.tile([128, E], F32, tag="tvals")
                nc.vector.tensor_add(tvals[:], psum[:], gum_sb[:])
                max8 = small.tile([128, 8], F32, tag="max8")
                nc.vector.max(max8[:], tvals[:])
                idx8 = small.tile([128, 8], U32, tag="idx8")
                nc.vector.max_index(idx8[:], max8[:], tvals[:])
                idx_i = small.tile([128, 1], I32, tag="idxi")
                nc.vector.tensor_copy(idx_i[:], idx8[:, 0:1])
                nc.sync.dma_start(out=idx_d[bsl, :], in_=idx_i[:])
                nc.vector.tensor_scalar(onehot[bt][:], tvals[:],
                                        max8[:, 0:1], None, op0=ALU.is_equal)

        # =========================== EXPERTS (f32r) ==========================
        NH = H // 128           # 32 h-tiles
        HB = H // 512           # 8 h-blocks (512 wide)
        NO = O // 512           # 2 o-blocks

        w1p = ctx.enter_context(tc.tile_pool(name="w1p", bufs=2))
        w2p = ctx.enter_context(tc.tile_pool(name="w2p", bufs=2))
        ehp = ctx.enter_context(tc.tile_pool(name="ehp", bufs=1))
        eop = ctx.enter_context(tc.tile_pool(name="eop", bufs=5))
        outp = ctx.enter_context(tc.tile_pool(name="outp", bufs=2))
        biasp = ctx.enter_context(tc.tile_pool(name="biasp", bufs=2))
        psA = ctx.enter_context(tc.tile_pool(name="psA", bufs=3, space="PSUM"))
        psB = ctx.enter_context(tc.tile_pool(name="psB", bufs=4, space="PSUM"))

        for e in range(E):
            # per-expert biases
            eb1_sb = biasp.tile([128, NH], F32, tag="eb1")
            nc.sync.dma_start(out=eb1_sb[:],
                              in_=eb1[e, :].rearrange("(t p) -> p t", p=128))
            eb2_sb = biasp.tile([128, O], F32, tag="eb2")
            nc.sync.dma_start(out=eb2_sb[:], in_=_bcast128(eb2[e:e + 1, :]))

            # ---- layer 1: eh[h, b] = relu(sum_d w1t[d, h] x[d, b]) ----
            eh_tiles = []
            for hb in range(HB):
                w1_sb = w1p.tile([128, D // 128, 512], F32R, tag="w1")
                if variant == "nodma":
                    nc.gpsimd.dma_start(
                        out=w1_sb[:, 0:1, 0:32],
                        in_=w1t[e, 0:128, hb * 512:hb * 512 + 32]
                            .rearrange("(c p) h -> p c h", p=128))
                else:
                    nc.gpsimd.dma_start(
                        out=w1_sb[:],
                        in_=w1t[e, :, hb * 512:(hb + 1) * 512]
                            .rearrange("(c p) h -> p c h", p=128))
                for ht in range(4):
                    h_idx = hb * 4 + ht
                    if variant != "nomm":
                        psum = psA.tile([128, BS], F32, tag="l1")
                        for dc in range(D // 128):
                            nc.tensor.matmul(psum[:],
                                             w1_sb[:, dc, ht * 128:(ht + 1) * 128],
                                             xr[dc][:],
                                             start=(dc == 0),
                                             stop=(dc == D // 128 - 1))
                        act_in = psum[:]
                    else:
                        act_in = w1_sb[:, 0, :]
                    eh_t = ehp.tile([128, BS], F32R, tag=f"eh{h_idx}")
                    nc.scalar.activation(out=eh_t[:], in_=act_in, func=AF.Relu,
                                         bias=eb1_sb[:, h_idx:h_idx + 1],
                                         scale=1.0)
                    eh_tiles.append(eh_t)

            if variant == "l1only":
                nc.sync.dma_start(out=final_d[0:128, 0:512],
                                  in_=eh_tiles[31][:].bitcast(F32))
                continue

            # ---- layer 2 + bias: eo_pre[b, o] = sum_h eh[h, b] w2t[h, o] ----
            eo_pre = [eop.tile([128, O], F32, tag="eo_pre", name=f"eo_pre{i}") for i in range(NB)]
            for ob in range(NO):
                osl = slice(ob * 512, (ob + 1) * 512)
                psums = ([psB.tile([128, 512], F32, tag="l2", name=f"l2ps{i}") for i in range(NB)]
                         if variant != "nomm" else None)
                for hg in range(HB):
                    w2_sb = w2p.tile([128, 4, 512], F32R, tag="w2")
                    if variant == "nodma":
                        nc.gpsimd.dma_start(
                            out=w2_sb[:, 0:1, 0:32],
                            in_=w2t[e, hg * 512:hg * 512 + 128,
                                    ob * 512:ob * 512 + 32]
                                .rearrange("(c p) o -> p c o", p=128))
                    else:
                        nc.gpsimd.dma_start(
                            out=w2_sb[:],
                            in_=w2t[e, hg * 512:(hg + 1) * 512, osl]
                                .rearrange("(c p) o -> p c o", p=128))
                    for bt in range(NB):
                        bsl = slice(bt * 128, (bt + 1) * 128)
                        for hc in range(4):
                            if variant == "nomm":
                                continue
                            nc.tensor.matmul(psums[bt][:],
                                             eh_tiles[hg * 4 + hc][:, bsl],
                                             w2_sb[:, hc, :],
                                             start=(hg == 0 and hc == 0),
                                             stop=(hg == HB - 1 and hc == 3))
                for bt in range(NB):
                    drain_in = (psums[bt][:] if variant != "nomm"
                                else eh_tiles[bt][:, 0:512].bitcast(F32))
                    nc.vector.tensor_add(eo_pre[bt][:, osl], drain_in,
                                         eb2_sb[:, osl])

            # ---- softmax over O + masked accumulation into final ----
            for bt in range(NB if variant != "nopost" else 0):
                bsl = slice(bt * 128, (bt + 1) * 128)
                negmax = small.tile([128, 1], F32, tag="negmax2")
                nc.vector.reduce_max(negmax[:], eo_pre[bt][:], axis=AX.X,
                                     negate=True)
                rsum = small.tile([128, 1], F32, tag="rsum2")
                nc.scalar.activation(out=eo_pre[bt][:], in_=eo_pre[bt][:],
                                     func=AF.Exp, bias=negmax[:], scale=1.0,
                                     accum_out=rsum[:])
                rinv = small.tile([128, 1], F32, tag="rinv2")
                nc.vector.reciprocal(rinv[:], rsum[:])
                eo_n = outp.tile([128, O], F32, tag="eo_n")
                nc.vector.tensor_scalar_mul(eo_n[:], eo_pre[bt][:], rinv[:])
                nc.sync.dma_start(out=eo_d[bsl, e, :], in_=eo_n[:])
                # final += onehot[:, e] * eo_n   (reuse eo_pre as scratch)
                nc.vector.tensor_scalar_mul(eo_pre[bt][:], eo_n[:],
                                            onehot[bt][:, e:e + 1])
                nc.vector.tensor_add(final_acc[bt][:], final_acc[bt][:],
                                     eo_pre[bt][:])

        for bt in range(NB if variant != "l1only" else 0):
            nc.sync.dma_start(out=final_d[bt * 128:(bt + 1) * 128, :],
                              in_=final_acc[bt][:])


_NC_CACHE = {}


def _get_nc(repeats: int = 1):
    if repeats not in _NC_CACHE:
        _NC_CACHE[repeats] = build(repeats)
    return _NC_CACHE[repeats]


def _gumbel_noise() -> np.ndarray:
    # Must reproduce jax.random.categorical(jax.random.key(42), ...) noise on
    # the same jax backend the reference runs on (RNG bits are backend-
    # dependent here), so use the default backend.
    import jax
    import jax.numpy as jnp
    g = jax.random.gumbel(jax.random.key(42), (B, E), jnp.float32)
    return np.asarray(g)


def _prep_inputs(inputs: dict) -> list[dict]:
    x = np.asarray(inputs["x"], np.float32)
    gw1 = np.asarray(inputs["gw1"], np.float32)
    gb1 = np.asarray(inputs["gb1"], np.float32)
    gw2 = np.asarray(inputs["gw2"], np.float32)
    gb2 = np.asarray(inputs["gb2"], np.float32)
    ew1 = np.asarray(inputs["ew1"], np.float32)
    eb1 = np.asarray(inputs["eb1"], np.float32)
    ew2 = np.asarray(inputs["ew2"], np.float32)
    eb2 = np.asarray(inputs["eb2"], np.float32)

    gw1t = np.ascontiguousarray(gw1.T)                # [D, GH]
    gw2t = np.ascontiguousarray(gw2.T)                # [GH, E]
    w1t = np.ascontiguousarray(ew1.transpose(0, 2, 1))  # [E, D, H]
    w2t = np.ascontiguousarray(ew2.transpose(0, 2, 1))  # [E, H, O]
    gum = _gumbel_noise()                             # [B, E]

    in_maps = []
    for c in range(NCORES):
        bsl = slice(c * BS, (c + 1) * BS)
        in_maps.append({
            "xt": np.ascontiguousarray(x[bsl].T),     # [D, BS]
            "gw1t": gw1t, "gb1": gb1, "gw2t": gw2t,
            "gb2": gb2.reshape(1, E),
            "w1t": w1t, "eb1": eb1, "w2t": w2t, "eb2": eb2,
            "gum": np.ascontiguousarray(gum[bsl]),
        })
    return in_maps


def run(inputs: dict, repeats: int = 1):
    nc = _get_nc(repeats)
    in_maps = _prep_inputs(inputs)
    res = run_bass_kernel_spmd(nc, in_maps, list(range(NCORES)))
    final = np.concatenate([r["final"] for r in res.results], axis=0)
    eo = np.concatenate([r["eo"] for r in res.results], axis=0)
    gate = np.concatenate([r["gate"] for r in res.results], axis=0)
    idx = np.concatenate([r["idx"][:, 0] for r in res.results], axis=0)
    return final, eo, gate, idx.astype(np.int32)


def kernel(**inputs):
    return run(inputs, repeats=1)
